# revision 1
# baseline (speedup 1.0000x reference)
"""Sparse-attention Bass kernel for Trainium2 (8 NeuronCores).

Problem (per batch element b of 8):
    scores = (q @ k^T) * scale            [2048, 2048]
    scores = where(mask[k], -1e9, scores)
    scores = scores * ratio[b]
    attn   = softmax(scores, axis=-1)
    out    = attn @ v                      [2048, 512]

Sharding: batch dim (8) -> one NeuronCore each (SPMD, same NEFF).

Device layout ("S^T layout"): scores are computed transposed,
S^T[k, q] = K @ Q^T (keys on partitions, queries on the free dim), so
  - the key-mask bias is a per-partition bias -> fused into the exp
    activation on the Scalar engine for free,
  - the AV matmul (contraction over keys) needs no transposes:
    lhsT = P^T tile [128k, 128q] (stationary), rhs = V [128k, 512d],
  - softmax denominators (sum over keys = partitions) come from a
    ones-vector matmul: rowsum[1, q] += ones[128,1].T @ P^T[128, q].

Normalization (divide by rowsum) is done on the host: the device returns
the unnormalized O = exp(S) @ V plus the row sums.
The scale*ratio[b] factor is folded into q on the host.

Written in raw Bass (explicit engine programs + semaphores): the walrus
build in this container allows at most ONE semaphore wait per
instruction, which the Tile scheduler's auto-generated waits violate.
Standalone wait_ge instructions sidestep the limit.

Engine roles:
  sync   (SP) : input DMAs (one HWDGE ring, FIFO -> one dma_sem)
  tensor (PE) : QK^T matmuls, rowsum matmuls, AV matmuls (float32r)
  scalar (ACT): exp (+mask bias), PSUM->SBUF copies, output DMAs
                (on ACT's own HWDGE ring so they don't queue behind
                the input DMAs)
"""

import sys

for _p in ("/opt/trn_rl_repo", "/opt/pypackages"):
    if _p not in sys.path:
        sys.path.append(_p)

import numpy as np
from contextlib import ExitStack

import concourse.bass as bass
from concourse import mybir

B, LQ, LK, D = 8, 2048, 2048, 512
P = 128
NCORES = 8
F32 = mybir.dt.float32
F32R = mybir.dt.float32r
NEG = np.float32(-1e9)

DT = D // P        # 4 d-tiles (contraction for QK^T)
KT = LK // P       # 16 key tiles (partitions of S^T)
QBS = 512          # queries per PSUM block (free dim of S^T)
QB = LQ // QBS     # 4 query superblocks
QTPB = QBS // P    # 4 query tiles (of 128) per superblock

# kq packing: [128, 16384] =
#   A(cols 0:2048)      kT d-tiles, keys 0:512
#   B(cols 2048:4096)   qT d-tiles, queries 0:512
#   C(cols 4096:10240)  kT d-tiles, keys 512:2048
#   D(cols 10240:16384) qT d-tiles, queries 512:2048
KQ_COLS = 4 * (LK + LQ) // P * P  # 16384
C0, D0 = 4096, 10240


def _kcol(d, j):
    """column of kq holding kT[d*128+p, j]"""
    return d * 512 + j if j < 512 else C0 + d * 1536 + (j - 512)


def _qcol(d, i):
    return 2048 + d * 512 + i if i < 512 else D0 + d * 1536 + (i - 512)


def _build_bass(niter=1):
    nc = bass.Bass()

    consts = nc.dram_tensor("consts", [P, KT], F32, kind="ExternalInput")
    onesd = nc.dram_tensor("onesd", [P, 1], F32R, kind="ExternalInput")
    kq = nc.dram_tensor("kq", [P, KQ_COLS], F32R, kind="ExternalInput")
    vv = nc.dram_tensor("vv", [P, KT * D], F32R, kind="ExternalInput")
    out_u = nc.dram_tensor("out_u", [LQ, D], F32, kind="ExternalOutput")
    sums = nc.dram_tensor("sums", [QB, QBS], F32, kind="ExternalOutput")

    EXP = mybir.ActivationFunctionType.Exp

    with ExitStack() as ctx:
        e = ctx.enter_context

        # SBUF
        sb_consts = e(nc.sbuf_tensor("sb_consts", [P, KT], F32))
        sb_ones = e(nc.sbuf_tensor("sb_ones", [P, 1], F32R))
        sb_kq = e(nc.sbuf_tensor("sb_kq", [P, KQ_COLS], F32R))
        sb_v = e(nc.sbuf_tensor("sb_v", [P, KT * D], F32R))
        # exp(S^T) tiles: [128k, 512q] per (qb parity, key tile)
        sb_pt = [
            [e(nc.sbuf_tensor(f"sb_pt{par}_{k}", [P, QBS], F32R)) for k in range(KT)]
            for par in range(2)
        ]
        sb_osb = [e(nc.sbuf_tensor(f"sb_osb{qt}", [P, D], F32)) for qt in range(QTPB)]
        sb_rs = [e(nc.sbuf_tensor(f"sb_rs{par}", [1, QBS], F32)) for par in range(2)]
        # per-partition partial sums of exp tiles (DVE), consumed by one
        # ones-matmul per block on PE
        sb_acc = [e(nc.sbuf_tensor(f"sb_acc{par}", [P, QBS], F32R)) for par in range(2)]

        # PSUM: 7 of 8 banks
        ps = [e(nc.psum_tensor(f"ps{i}", [P, QBS], F32)) for i in range(4)]
        po = [e(nc.psum_tensor(f"po{i}", [P, D], F32)) for i in range(2)]
        rs = [e(nc.psum_tensor(f"rs{i}", [P, QBS], F32)) for i in range(2)]

        # one semaphore per input DMA: HWDGE DMAs on one ring may
        # complete out of order, so a shared counter can't identify which
        # transfer landed
        s_consts = e(nc.semaphore("s_consts"))
        s_ones = e(nc.semaphore("s_ones"))
        s_ab = e(nc.semaphore("s_ab"))
        s_c = [e(nc.semaphore(f"s_c{i}")) for i in range(3)]
        s_d = [e(nc.semaphore(f"s_d{i}")) for i in range(3)]
        s_v = [e(nc.semaphore(f"s_v{i}")) for i in range(4)]
        # per-output-buffer DMA-completion semaphores (buffer reuse gates)
        s_osb = [e(nc.semaphore(f"s_osb{qt}")) for qt in range(QTPB)]
        s_rsb = [e(nc.semaphore(f"s_rsb{par}")) for par in range(2)]
        pe_sem = e(nc.semaphore("pe_sem"))
        act_sem = e(nc.semaphore("act_sem"))
        dve_sem = e(nc.semaphore("dve_sem"))

        # ---- semaphore tick bookkeeping ----
        # gb = global block index (niter * QB blocks total); data block
        # qb = gb % QB.
        # pe_sem increments: per gb: 16 QK-group finals, 1 rowsum final,
        # 4 AV finals = 21
        def tick_qk(gb, k):
            return gb * 21 + k + 1

        def tick_av(gb, qt):
            # PE order per block: 16 QK groups, AV qt=0, rowsum MM, AV qt=1..3
            return gb * 21 + (17 if qt == 0 else 18 + qt)

        def tick_rs(gb):
            return gb * 21 + 18

        def tick_acc(gb):
            # dve_sem: 15 accumulate-adds per block
            return 15 * (gb + 1)

        # act_sem increments: per gb: 16 exps, 1 rs copy, 4 po copies = 21
        def tick_exp(gb, k):
            return gb * 21 + k + 1

        def tick_rsc(gb):
            return gb * 21 + 17

        def tick_poc(gb, qt):
            return gb * 21 + 18 + qt

        with nc.Block() as block:

            @block.sync
            def _(sync):
                # issue order == consumption order so the PE rarely starves:
                # consts/ones, AB, C by key-range, V by key-group, D by
                # query-block
                sync.dma_start(sb_consts[:, :], consts[:, :]).then_inc(s_consts, 16)
                sync.dma_start(sb_ones[:, :], onesd[:, :]).then_inc(s_ones, 16)
                sync.dma_start(sb_kq[:, 0:C0], kq[:, 0:C0]).then_inc(s_ab, 16)

                def kq3d(t, base):
                    # view of the C or D region as [128, d=4, 1536]
                    return t[:, base:base + 6144].rearrange(
                        "p (d j) -> p d j", d=4
                    )
                for i in range(3):
                    js = slice(i * 512, (i + 1) * 512)
                    sync.dma_start(
                        kq3d(sb_kq, C0)[:, :, js], kq3d(kq, C0)[:, :, js]
                    ).then_inc(s_c[i], 16)
                for i in range(4):
                    cs = slice(i * 4 * D, (i + 1) * 4 * D)
                    sync.dma_start(sb_v[:, cs], vv[:, cs]).then_inc(s_v[i], 16)
                for i in range(3):
                    js = slice(i * 512, (i + 1) * 512)
                    sync.dma_start(
                        kq3d(sb_kq, D0)[:, :, js], kq3d(kq, D0)[:, :, js]
                    ).then_inc(s_d[i], 16)

            @block.tensor
            def _(tensor):
                last_wait = {}  # sem name -> value already waited for

                def wait(sem, val, name):
                    if val > last_wait.get(name, -1):
                        tensor.wait_ge(sem, val)
                        last_wait[name] = val

                for gb in range(niter * QB):
                    qb = gb % QB
                    # ---- QK^T + rowsum phase ----
                    for k in range(KT):
                        g = gb * KT + k  # global k-iteration index
                        # input availability
                        if qb == 0:
                            if k < 4:
                                wait(s_ab, 16, "ab")
                                wait(s_ones, 16, "ones")
                            else:
                                wait(s_c[k // 4 - 1], 16, f"c{k // 4 - 1}")
                        else:
                            wait(s_d[qb - 1], 16, f"d{qb - 1}")
                        # ps[g%4] must have been consumed by exp of g-4.
                        # stride 2: waiting for exp(g-3) covers groups g and
                        # g+1 with one instruction, and exp(g-3) is ~2.5
                        # groups in the past so the wait never stalls
                        if g >= 4 and g % 2 == 0:
                            g3 = g - 3
                            wait(act_sem, tick_exp(g3 // KT, g3 % KT), "act")
                        for d in range(DT):
                            mm = tensor.matmul(
                                ps[g % 4][:, :],
                                lhsT=sb_kq[:, _kcol(d, k * P):_kcol(d, k * P) + P],
                                rhs=sb_kq[:, _qcol(d, qb * QBS):_qcol(d, qb * QBS) + QBS],
                                start=(d == 0),
                                stop=(d == DT - 1),
                            )
                            if d == DT - 1:
                                mm.then_inc(pe_sem, 1)

                    # ---- AV phase ----
                    for qt in range(QTPB):
                        # po[qt%2] consumed by copy of (gb,qt-2) / (gb-1,qt+2)
                        if qt >= 2:
                            wait(act_sem, tick_poc(gb, qt - 2), "act")
                        elif gb >= 1:
                            wait(act_sem, tick_poc(gb - 1, qt + 2), "act")
                        if qt == 0:
                            # exps 0..13 are long done by now (ACT trails the
                            # QK phase by ~1 tile); one wait covers them
                            wait(act_sem, tick_exp(gb, KT - 3), "act")
                        for k in range(KT):
                            if qt == 0:
                                if k >= KT - 2:
                                    wait(act_sem, tick_exp(gb, k), "act")
                                wait(s_v[k // 4], 16, f"v{k // 4}")
                            mm = tensor.matmul(
                                po[qt % 2][:, :],
                                lhsT=sb_pt[gb % 2][k][:, qt * P:(qt + 1) * P],
                                rhs=sb_v[:, k * D:(k + 1) * D],
                                start=(k == 0),
                                stop=(k == KT - 1),
                            )
                            if k == KT - 1:
                                mm.then_inc(pe_sem, 1)
                        if qt == 0:
                            # single partition-reduction matmul over the
                            # DVE-accumulated exp sums
                            wait(dve_sem, tick_acc(gb), "dve")
                            tensor.matmul(
                                rs[gb % 2][0:1, :],
                                lhsT=sb_ones[:, :],
                                rhs=sb_acc[gb % 2][:, :],
                                start=True,
                                stop=True,
                            ).then_inc(pe_sem, 1)

            @block.vector
            def _(vector):
                last_wait = {}

                def wait(sem, val, name):
                    if val > last_wait.get(name, -1):
                        vector.wait_ge(sem, val)
                        last_wait[name] = val

                ndve = 0
                for gb in range(niter * QB):
                    # acc[gb%2] readable again after PE's rowsum MM of gb-2
                    if gb >= 2:
                        wait(pe_sem, tick_rs(gb - 2), "pe")
                    for j in range(1, KT):
                        wait(act_sem, tick_exp(gb, j), "act")
                        if j > 1:
                            # same-engine RAW on acc: wait for own pipe drain
                            wait(dve_sem, ndve, "dve")
                        vector.tensor_add(
                            sb_acc[gb % 2][:, :],
                            sb_pt[gb % 2][0][:, :] if j == 1 else sb_acc[gb % 2][:, :],
                            sb_pt[gb % 2][j][:, :],
                        ).then_inc(dve_sem, 1)
                        ndve += 1

            @block.scalar
            def _(scalar):
                last_wait = {}

                def wait(sem, val, name):
                    if val > last_wait.get(name, -1):
                        scalar.wait_ge(sem, val)
                        last_wait[name] = val

                wait(s_consts, 16, "consts")
                for gb in range(niter * QB):
                    qb = gb % QB
                    for k in range(KT):
                        g = gb * KT + k
                        wait(pe_sem, tick_qk(gb, k), "pe")
                        scalar.activation(
                            sb_pt[gb % 2][k][:, :],
                            ps[g % 4][:, :],
                            EXP,
                            bias=sb_consts[:, k:k + 1],
                            scale=1.0,
                        ).then_inc(act_sem, 1)
                    # rowsum copy + DMA (ACT's own HWDGE ring)
                    if gb >= 2:
                        wait(s_rsb[gb % 2], 16 * (gb // 2), f"rsb{gb % 2}")
                    wait(pe_sem, tick_rs(gb), "pe")
                    scalar.copy(sb_rs[gb % 2][:, :], rs[gb % 2][0:1, :]).then_inc(
                        act_sem, 1
                    )
                    # self-wait: the DMA engine reads sb_rs asynchronously,
                    # so the copy must have fully drained first
                    wait(act_sem, tick_rsc(gb), "act")
                    scalar.dma_start(
                        sums[qb:qb + 1, :], sb_rs[gb % 2][:, :]
                    ).then_inc(s_rsb[gb % 2], 16)
                    # AV output copies + DMAs
                    for qt in range(QTPB):
                        if gb >= 1:
                            wait(s_osb[qt], 16 * gb, f"osb{qt}")
                        wait(pe_sem, tick_av(gb, qt), "pe")
                        scalar.copy(sb_osb[qt][:, :], po[qt % 2][:, :]).then_inc(
                            act_sem, 1
                        )
                        wait(act_sem, tick_poc(gb, qt), "act")
                        row = (qb * QTPB + qt) * P
                        scalar.dma_start(
                            out_u[row:row + P, :], sb_osb[qt][:, :]
                        ).then_inc(s_osb[qt], 16)
                # drain: all output DMAs landed
                for qt in range(QTPB):
                    scalar.wait_ge(s_osb[qt], 16 * QB * niter)
                for par in range(2):
                    scalar.wait_ge(s_rsb[par], 16 * 2 * niter)

    return nc


_NC_CACHE = {}


def _get_nc(niter=1):
    if niter not in _NC_CACHE:
        _NC_CACHE[niter] = _build_bass(niter)
    return _NC_CACHE[niter]


_RUNNER_CACHE = {}


def _get_runner():
    """Compile once, reuse across kernel() calls. Returns a callable
    taking concatenated input arrays and returning (out_u, sums) stacked
    per core."""
    if "runner" in _RUNNER_CACHE:
        return _RUNNER_CACHE["runner"]

    import jax
    from jax.sharding import Mesh, PartitionSpec, NamedSharding
    from jax.experimental.shard_map import shard_map
    from concourse.bass2jax import (
        _bass_exec_p, install_neuronx_cc_hook, partition_id_tensor,
    )

    nc = _get_nc()
    install_neuronx_cc_hook()
    in_names = []
    out_names = []
    out_avals = []
    zero_like = []
    part_name = nc.partition_id_tensor.name if nc.partition_id_tensor else None
    for alloc in nc.m.functions[0].allocations:
        if not isinstance(alloc, mybir.MemoryLocationSet):
            continue
        name = alloc.memorylocations[0].name
        if alloc.kind == "ExternalInput":
            if name != part_name:
                in_names.append(name)
        elif alloc.kind == "ExternalOutput":
            np_dt = mybir.dt.np(alloc.dtype)
            out_avals.append(jax.core.ShapedArray(tuple(alloc.tensor_shape), np_dt))
            out_names.append(name)
            zero_like.append((tuple(alloc.tensor_shape), np_dt))
    n_params = len(in_names)
    bind_in_names = tuple(in_names + out_names + ([part_name] if part_name else []))

    def _body(*args):
        ins = list(args[:n_params])
        outs = list(args[n_params:])
        extra = [partition_id_tensor()] if part_name else []
        outs = list(_bass_exec_p.bind(
            *ins, *outs, *extra,
            out_avals=tuple(out_avals),
            in_names=bind_in_names,
            out_names=tuple(out_names),
            lowering_input_output_aliases=(),
            sim_require_finite=True,
            sim_require_nnan=True,
            nc=nc,
        ))
        return tuple(outs)

    devices = jax.devices()[:NCORES]
    mesh = Mesh(np.asarray(devices), ("core",))
    n_outs = len(out_names)
    sharded = jax.jit(
        shard_map(
            _body, mesh=mesh,
            in_specs=(PartitionSpec("core"),) * (n_params + n_outs),
            out_specs=(PartitionSpec("core"),) * n_outs,
            check_rep=False,
        ),
        donate_argnums=tuple(range(n_params, n_params + n_outs)),
        keep_unused=True,
    )

    sh = NamedSharding(mesh, PartitionSpec("core"))
    import jax.numpy as jnp
    zeros_fn = jax.jit(
        lambda: tuple(
            jnp.zeros((NCORES * s[0],) + s[1:], d) for s, d in zero_like
        ),
        out_shardings=(sh,) * n_outs,
    )

    def run(per_core):
        # [8, s0, ...] -> [8*s0, ...] is a reshape view, not a copy
        concat_in = [
            np.ascontiguousarray(per_core[n]).reshape(
                (NCORES * per_core[n].shape[1],) + tuple(per_core[n].shape[2:])
            )
            for n in in_names
        ]
        # donated output buffers created on-device: avoids shipping 32 MB
        # of zeros over the (slow) axon link every call
        zeros = zeros_fn()
        outs = sharded(*concat_in, *zeros)
        res = {}
        for i, name in enumerate(out_names):
            a = np.asarray(outs[i])
            res[name] = a.reshape(NCORES, *out_avals[i].shape)
        return res

    _RUNNER_CACHE["runner"] = run
    return run


def _pack_inputs(q, k, v, ratio, scale, attn_mask):
    """Host-side packing into the per-core flat layouts."""
    mult = np.float32(scale) * ratio.astype(np.float32)  # [B]
    qs = q.astype(np.float32) * mult[:, None, None]      # [B, LQ, D]

    # kd[b, d, p, j] = k[b, j, d*128+p]
    kd = np.ascontiguousarray(k.astype(np.float32).transpose(0, 2, 1)).reshape(
        B, DT, P, LK
    )
    qd = np.ascontiguousarray(qs.transpose(0, 2, 1)).reshape(B, DT, P, LQ)

    def reg(x, lo, hi):  # [B, DT, P, hi-lo] -> [B, P, DT*(hi-lo)]
        r = x[:, :, :, lo:hi].transpose(0, 2, 1, 3)
        return np.ascontiguousarray(r).reshape(B, P, -1)

    kq = np.concatenate(
        [reg(kd, 0, 512), reg(qd, 0, 512), reg(kd, 512, LK), reg(qd, 512, LQ)],
        axis=2,
    )  # [B, 128, 16384]

    vvl = (
        v.astype(np.float32).reshape(B, KT, P, D).transpose(0, 2, 1, 3)
    )
    vvl = np.ascontiguousarray(vvl).reshape(B, P, KT * D)

    # consts[b, p, k] = mask bias for key k*128+p
    bias = np.where(attn_mask, NEG, np.float32(0.0)).astype(np.float32)
    consts = np.ascontiguousarray(bias.reshape(B, KT, P).transpose(0, 2, 1))

    ones = np.ones((P, 1), dtype=np.float32)
    return kq, vvl, consts, ones


def kernel(q, k, v, ratio, scale, attn_mask):
    """Full inputs in, full output out. Shards batch across 8 cores."""
    q = np.asarray(q)
    k = np.asarray(k)
    v = np.asarray(v)
    ratio = np.asarray(ratio)
    scale = np.asarray(scale)
    attn_mask = np.asarray(attn_mask)
    assert q.shape == (B, LQ, D) and k.shape == (B, LK, D)
    run = _get_runner()
    kq, vvl, consts, ones = _pack_inputs(q, k, v, ratio, scale, attn_mask)
    per_core = {
        "kq": kq, "vv": vvl, "consts": consts,
        "onesd": np.broadcast_to(ones, (B,) + ones.shape),
    }
    res = run(per_core)
    out_un = res["out_u"]                          # [B, LQ, D]
    ssum = res["sums"].reshape(B, LQ)
    out = out_un / ssum[:, :, None]
    return out.astype(np.float32)



# revision 31
# speedup vs baseline: 1.4137x; 1.4137x over previous
"""Sparse-attention Bass kernel for Trainium2 (8 NeuronCores).

Problem (per batch element b of 8):
    scores = (q @ k^T) * scale            [2048, 2048]
    scores = where(mask[k], -1e9, scores)
    scores = scores * ratio[b]
    attn   = softmax(scores, axis=-1)
    out    = attn @ v                      [2048, 512]

Sharding: batch dim (8) -> one NeuronCore each (SPMD, same NEFF).

Key compaction: masked keys (~10%) contribute exactly 0 to both the
softmax numerator and denominator (exp(-1e9) == 0 in f32), so the host
drops them and packs only the kept keys, padded to a multiple of 128
with -1e9-bias slots. 15 key tiles instead of 16 -> 6.25% less PE work.

Device layout ("S^T layout"): scores are computed transposed,
S^T[k, q] = K @ Q^T (keys on partitions, queries on the free dim), so
  - the pad-slot bias is a per-partition bias -> fused into the exp
    activation on the Scalar engine for free,
  - the AV matmul (contraction over keys) needs no transposes:
    lhsT = P^T tile [128k, 128q] (stationary), rhs = V [128k, 512d],
  - softmax denominators (sum over keys = partitions) come from a
    ones-vector matmul over DVE-accumulated partials.

All matmul operands are bf16 (inputs quantized on host, exp output
written as bf16 by the Scalar engine): same 1 row/cycle PE stream rate
as float32r, but weight loads are 2-4x faster (FWL), shrinking the
per-matmul weight-switch overhead. Accumulation stays fp32 in PSUM;
rowsum accumulation stays fp32 on DVE. End-to-end error ~0.4% rms,
well inside the 2e-2 gate.

Normalization (divide by rowsum) is done on the host: the device returns
the unnormalized O = exp(S) @ V plus the row sums.
The scale*ratio[b] factor is folded into q on the host.

Written in raw Bass (explicit engine programs + semaphores): the walrus
build in this container allows at most ONE semaphore wait per
instruction, which the Tile scheduler's auto-generated waits violate.
Standalone wait_ge instructions sidestep the limit.

Engine roles:
  sync   (SP) : input DMAs (one HWDGE ring, FIFO -> per-chunk sems)
  tensor (PE) : QK^T matmuls, rowsum matmuls, AV matmuls (bf16)
  vector (DVE): partial rowsum accumulation (fp32)
  scalar (ACT): exp (+pad bias) -> bf16, PSUM->SBUF copies, output DMAs
                (on ACT's own HWDGE ring so they don't queue behind
                the input DMAs)
"""

import sys

for _p in ("/opt/trn_rl_repo", "/opt/pypackages"):
    if _p not in sys.path:
        sys.path.append(_p)

import numpy as np
from contextlib import ExitStack

import concourse.bass as bass
from concourse import mybir

B, LQ, LK, D = 8, 2048, 2048, 512
P = 128
NCORES = 8
F32 = mybir.dt.float32
F32R = mybir.dt.float32r
BF16 = mybir.dt.bfloat16
NPBF16 = mybir.dt.np(BF16)
NEG = np.float32(-1e9)

DT = D // P        # 4 d-tiles (contraction for QK^T)
QBS = 512          # queries per PSUM block (free dim of S^T)
QB = LQ // QBS     # 4 query superblocks
QTPB = QBS // P    # 4 query tiles (of 128) per superblock

DEFAULT_KT = 15    # key tiles after compaction (keys padded to KT*128)


def _build_bass(niter=1, kt=DEFAULT_KT):
    KT = kt
    KTP = KT * P           # packed key count
    KOFF = DT * KTP        # start of the Q region in kq
    KQ_COLS = KOFF + DT * LQ

    nc = bass.Bass()

    consts = nc.dram_tensor("consts", [P, KT], F32, kind="ExternalInput")
    onesd = nc.dram_tensor("onesd", [P, 1], BF16, kind="ExternalInput")
    # kq packing (bf16): K region cols [0, KOFF): col = d*KTP + key
    #                    Q region cols [KOFF, ...): col = KOFF + d*LQ + q
    kq = nc.dram_tensor("kq", [P, KQ_COLS], BF16, kind="ExternalInput")
    # v: partition p = key kt*128+p, col = kt*D + d
    vv = nc.dram_tensor("vv", [P, KT * D], BF16, kind="ExternalInput")
    out_u = nc.dram_tensor("out_u", [LQ, D], F32, kind="ExternalOutput")
    sums = nc.dram_tensor("sums", [QB, QBS], F32, kind="ExternalOutput")

    EXP = mybir.ActivationFunctionType.Exp

    with ExitStack() as ctx:
        e = ctx.enter_context

        # SBUF
        sb_consts = e(nc.sbuf_tensor("sb_consts", [P, KT], F32))
        sb_ones = e(nc.sbuf_tensor("sb_ones", [P, 1], BF16))
        sb_kq = e(nc.sbuf_tensor("sb_kq", [P, KQ_COLS], BF16))
        sb_v = e(nc.sbuf_tensor("sb_v", [P, KT * D], BF16))
        # exp(S^T) tiles: [128k, 512q] per (qb parity, key tile)
        sb_pt = [
            [e(nc.sbuf_tensor(f"sb_pt{par}_{k}", [P, QBS], BF16)) for k in range(KT)]
            for par in range(2)
        ]
        sb_osb = [e(nc.sbuf_tensor(f"sb_osb{qt}", [P, D], F32)) for qt in range(QTPB)]
        sb_rs = [e(nc.sbuf_tensor(f"sb_rs{par}", [1, QBS], F32)) for par in range(2)]
        # per-partition partial sums of exp tiles (DVE, fp32), rounded to
        # bf16 once at the end so the ones-matmul runs at bf16 rate
        sb_acc = [e(nc.sbuf_tensor(f"sb_acc{par}", [P, QBS], F32)) for par in range(2)]
        sb_accb = [e(nc.sbuf_tensor(f"sb_accb{par}", [P, QBS], BF16)) for par in range(2)]
        sb_tmp = e(nc.sbuf_tensor("sb_tmp", [P, QBS], F32))

        # PSUM: 8 banks
        ps = [e(nc.psum_tensor(f"ps{i}", [P, QBS], F32)) for i in range(4)]
        po = [e(nc.psum_tensor(f"po{i}", [P, D], F32)) for i in range(2)]
        rs = [e(nc.psum_tensor(f"rs{i}", [P, QBS], F32)) for i in range(2)]

        # one semaphore per input DMA chunk: HWDGE DMAs on one ring may
        # complete out of order, so a shared counter can't identify which
        # transfer landed
        s_consts = e(nc.semaphore("s_consts"))
        s_ones = e(nc.semaphore("s_ones"))
        s_ab = e(nc.semaphore("s_ab"))
        s_c = [e(nc.semaphore(f"s_c{i}")) for i in range(3)]
        s_d = [e(nc.semaphore(f"s_d{i}")) for i in range(3)]
        s_v = [e(nc.semaphore(f"s_v{i}")) for i in range(4)]
        # per-output-buffer DMA-completion semaphores (buffer reuse gates)
        s_osb = [e(nc.semaphore(f"s_osb{qt}")) for qt in range(QTPB)]
        s_rsb = [e(nc.semaphore(f"s_rsb{par}")) for par in range(2)]
        pe_sem = e(nc.semaphore("pe_sem"))
        act_sem = e(nc.semaphore("act_sem"))
        dve_sem = e(nc.semaphore("dve_sem"))

        # ---- semaphore tick bookkeeping ----
        # gb = global block index (niter * QB blocks total); data block
        # qb = gb % QB.
        # pe_sem increments per gb: KT QK-group finals, 1 rowsum final,
        # 4 AV finals = KT + 5.
        # PE order per block: KT QK groups, AV qt0, rowsum MM, AV qt1-3.
        PEB = KT + 5
        # act_sem order per block: KT exps, rs copy, po0..po3 copies
        ACB = KT + 5

        def tick_qk(gb, k):
            return gb * PEB + k + 1

        def tick_av(gb, qt):
            return gb * PEB + (KT + 1 if qt == 0 else KT + 2 + qt)

        def tick_rs(gb):
            return gb * PEB + KT + 2

        def tick_acc(gb):
            # dve_sem: KT-1 accumulate-adds per block (KT >= 2)
            return max(KT - 1, 0) * (gb + 1)

        def tick_exp(gb, k):
            return gb * ACB + k + 1

        def tick_rsc(gb):
            return gb * ACB + KT + 1

        def tick_poc(gb, qt):
            return gb * ACB + KT + 2 + qt

        # K-chunk boundaries (in key-tile units) for the 3 "C" DMAs:
        # tiles 4..KT-1 split as evenly as possible into 3 chunks
        c_bounds = [4 + ((KT - 4) * i) // 3 for i in range(4)]  # e.g. [4,7,11,15]

        def c_idx_for_tile(k):  # which s_c gates key tile k (k >= 4)
            for i in range(3):
                if k < c_bounds[i + 1]:
                    return i
            return 2

        # V quarters (in key-tile units)
        v_bounds = [(KT * i) // 4 for i in range(5)]

        def v_idx_for_tile(k):
            for i in range(4):
                if k < v_bounds[i + 1]:
                    return i
            return 3

        with nc.Block() as block:

            @block.sync
            def _(sync):
                # issue order == consumption order so the PE rarely starves:
                # consts/ones, K(tiles 0-3), Q(block 0), K rest by range,
                # V by quarter, Q blocks 1-3
                sync.dma_start(sb_consts[:, :], consts[:, :]).then_inc(s_consts, 16)
                sync.dma_start(sb_ones[:, :], onesd[:, :]).then_inc(s_ones, 16)

                def k3d(t):  # view of the K region as [128, d=4, KTP]
                    return t[:, 0:KOFF].rearrange("p (d j) -> p d j", d=DT)

                def q3d(t):  # view of the Q region as [128, d=4, LQ]
                    return t[:, KOFF:KOFF + DT * LQ].rearrange(
                        "p (d j) -> p d j", d=DT
                    )

                sync.dma_start(
                    k3d(sb_kq)[:, :, 0:4 * P], k3d(kq)[:, :, 0:4 * P]
                ).then_inc(s_ab, 16)
                sync.dma_start(
                    q3d(sb_kq)[:, :, 0:QBS], q3d(kq)[:, :, 0:QBS]
                ).then_inc(s_ab, 16)
                for i in range(3):
                    if c_bounds[i] == c_bounds[i + 1]:
                        continue  # empty chunk (small KT); never waited on
                    js = slice(c_bounds[i] * P, c_bounds[i + 1] * P)
                    sync.dma_start(
                        k3d(sb_kq)[:, :, js], k3d(kq)[:, :, js]
                    ).then_inc(s_c[i], 16)
                for i in range(4):
                    if v_bounds[i] == v_bounds[i + 1]:
                        continue
                    cs = slice(v_bounds[i] * D, v_bounds[i + 1] * D)
                    sync.dma_start(sb_v[:, cs], vv[:, cs]).then_inc(s_v[i], 16)
                for i in range(3):
                    js = slice((i + 1) * QBS, (i + 2) * QBS)
                    sync.dma_start(
                        q3d(sb_kq)[:, :, js], q3d(kq)[:, :, js]
                    ).then_inc(s_d[i], 16)

            @block.tensor
            def _(tensor):
                last_wait = {}  # sem name -> value already waited for

                def wait(sem, val, name):
                    if val > last_wait.get(name, -1):
                        tensor.wait_ge(sem, val)
                        last_wait[name] = val

                for gb in range(niter * QB):
                    qb = gb % QB
                    # ---- QK^T phase ----
                    for k in range(KT):
                        g = gb * KT + k  # global k-iteration index
                        # input availability
                        if qb == 0:
                            if k < 4:
                                wait(s_ab, 32, "ab")
                            else:
                                ci = c_idx_for_tile(k)
                                wait(s_c[ci], 16, f"c{ci}")
                        else:
                            wait(s_d[qb - 1], 16, f"d{qb - 1}")
                        # ps[g%4] must have been consumed by exp of g-4.
                        # stride 2: waiting for exp(g-3) covers groups g and
                        # g+1 with one instruction, and exp(g-3) is ~2.5
                        # groups in the past so the wait never stalls
                        if g >= 4 and g % 2 == 0:
                            g3 = g - 3
                            wait(act_sem, tick_exp(g3 // KT, g3 % KT), "act")
                        for d in range(DT):
                            col = d * KTP + k * P
                            mm = tensor.matmul(
                                ps[g % 4][:, :],
                                lhsT=sb_kq[:, col:col + P],
                                rhs=sb_kq[
                                    :, KOFF + d * LQ + qb * QBS:
                                    KOFF + d * LQ + qb * QBS + QBS
                                ],
                                start=(d == 0),
                                stop=(d == DT - 1),
                            )
                            if d == DT - 1:
                                mm.then_inc(pe_sem, 1)

                    # ---- AV phase ----
                    for qt in range(QTPB):
                        # po[qt%2] consumed by copy of (gb,qt-2) / (gb-1,qt+2)
                        if qt >= 2:
                            wait(act_sem, tick_poc(gb, qt - 2), "act")
                        elif gb >= 1:
                            wait(act_sem, tick_poc(gb - 1, qt + 2), "act")
                        if qt == 0:
                            # exps 0..KT-3 are long done by now (ACT trails
                            # the QK phase by ~1 tile); one wait covers them
                            wait(act_sem, tick_exp(gb, max(KT - 3, 0)), "act")
                        for k in range(KT):
                            if qt == 0:
                                if k >= KT - 2:
                                    wait(act_sem, tick_exp(gb, k), "act")
                                vi = v_idx_for_tile(k)
                                wait(s_v[vi], 16, f"v{vi}")
                            mm = tensor.matmul(
                                po[qt % 2][:, :],
                                lhsT=sb_pt[gb % 2][k][:, qt * P:(qt + 1) * P],
                                rhs=sb_v[:, k * D:(k + 1) * D],
                                start=(k == 0),
                                stop=(k == KT - 1),
                            )
                            if k == KT - 1:
                                mm.then_inc(pe_sem, 1)
                        if qt == 0:
                            # single partition-reduction matmul over the
                            # DVE-accumulated exp sums
                            wait(s_ones, 16, "ones")
                            if KT > 1:
                                wait(dve_sem, tick_acc(gb), "dve")
                                rs_rhs = sb_accb[gb % 2][:, :]
                            else:
                                rs_rhs = sb_pt[gb % 2][0][:, :]
                            tensor.matmul(
                                rs[gb % 2][0:1, :],
                                lhsT=sb_ones[:, :],
                                rhs=rs_rhs,
                                start=True,
                                stop=True,
                            ).then_inc(pe_sem, 1)

            @block.vector
            def _(vector):
                last_wait = {}

                def wait(sem, val, name):
                    if val > last_wait.get(name, -1):
                        vector.wait_ge(sem, val)
                        last_wait[name] = val

                ndve = 0
                for gb in range(niter * QB):
                    # accb[gb%2] readable again after PE's rowsum MM of gb-2
                    if gb >= 2:
                        wait(pe_sem, tick_rs(gb - 2), "pe")
                    par = gb % 2
                    if KT == 2:
                        wait(act_sem, tick_exp(gb, 1), "act")
                        vector.tensor_add(
                            sb_accb[par][:, :],
                            sb_pt[par][0][:, :], sb_pt[par][1][:, :],
                        ).then_inc(dve_sem, 1)
                        ndve += 1
                    elif KT == 3:
                        wait(act_sem, tick_exp(gb, 1), "act")
                        vector.tensor_add(
                            sb_acc[par][:, :],
                            sb_pt[par][0][:, :], sb_pt[par][1][:, :],
                        ).then_inc(dve_sem, 1)
                        ndve += 1
                        wait(act_sem, tick_exp(gb, 2), "act")
                        wait(dve_sem, ndve, "dve")
                        vector.tensor_add(
                            sb_accb[par][:, :],
                            sb_acc[par][:, :], sb_pt[par][2][:, :],
                        ).then_inc(dve_sem, 1)
                        ndve += 1
                    elif KT >= 4:
                        # fp32 accumulation of pt0..pt[KT-3] into acc, the
                        # last two tiles into tmp, then a single bf16
                        # rounding: accb = acc + tmp
                        for j in range(1, KT - 2):
                            wait(act_sem, tick_exp(gb, j), "act")
                            if j > 1:
                                # same-engine RAW on acc: wait for own pipe
                                # drain
                                wait(dve_sem, ndve, "dve")
                            vector.tensor_add(
                                sb_acc[par][:, :],
                                sb_pt[par][0][:, :] if j == 1
                                else sb_acc[par][:, :],
                                sb_pt[par][j][:, :],
                            ).then_inc(dve_sem, 1)
                            ndve += 1
                        wait(act_sem, tick_exp(gb, KT - 1), "act")
                        vector.tensor_add(
                            sb_tmp[:, :],
                            sb_pt[par][KT - 2][:, :],
                            sb_pt[par][KT - 1][:, :],
                        ).then_inc(dve_sem, 1)
                        ndve += 1
                        wait(dve_sem, ndve, "dve")
                        vector.tensor_add(
                            sb_accb[par][:, :],
                            sb_acc[par][:, :], sb_tmp[:, :],
                        ).then_inc(dve_sem, 1)
                        ndve += 1

            @block.scalar
            def _(scalar):
                last_wait = {}

                def wait(sem, val, name):
                    if val > last_wait.get(name, -1):
                        scalar.wait_ge(sem, val)
                        last_wait[name] = val

                wait(s_consts, 16, "consts")
                for gb in range(niter * QB):
                    qb = gb % QB
                    for k in range(KT):
                        g = gb * KT + k
                        wait(pe_sem, tick_qk(gb, k), "pe")
                        scalar.activation(
                            sb_pt[gb % 2][k][:, :],
                            ps[g % 4][:, :],
                            EXP,
                            bias=sb_consts[:, k:k + 1],
                            scale=1.0,
                        ).then_inc(act_sem, 1)
                    # rowsum copy + DMA (ACT's own HWDGE ring)
                    if gb >= 2:
                        wait(s_rsb[gb % 2], 16 * (gb // 2), f"rsb{gb % 2}")
                    wait(pe_sem, tick_rs(gb), "pe")
                    scalar.copy(sb_rs[gb % 2][:, :], rs[gb % 2][0:1, :]).then_inc(
                        act_sem, 1
                    )
                    # self-wait: the DMA engine reads sb_rs asynchronously,
                    # so the copy must have fully drained first
                    wait(act_sem, tick_rsc(gb), "act")
                    scalar.dma_start(
                        sums[qb:qb + 1, :], sb_rs[gb % 2][:, :]
                    ).then_inc(s_rsb[gb % 2], 16)
                    # AV output copies + DMAs
                    for qt in range(QTPB):
                        if gb >= 1:
                            wait(s_osb[qt], 16 * gb, f"osb{qt}")
                        wait(pe_sem, tick_av(gb, qt), "pe")
                        scalar.copy(sb_osb[qt][:, :], po[qt % 2][:, :]).then_inc(
                            act_sem, 1
                        )
                        wait(act_sem, tick_poc(gb, qt), "act")
                        row = (qb * QTPB + qt) * P
                        scalar.dma_start(
                            out_u[row:row + P, :], sb_osb[qt][:, :]
                        ).then_inc(s_osb[qt], 16)
                # drain: all output DMAs landed
                for qt in range(QTPB):
                    scalar.wait_ge(s_osb[qt], 16 * QB * niter)
                for par in range(2):
                    scalar.wait_ge(s_rsb[par], 16 * 2 * niter)

    return nc


_NC_CACHE = {}

# KT used by the most recent _pack_inputs call; _get_nc defaults to it so
# the pack -> compile -> run sequence stays consistent.
_CUR_KT = DEFAULT_KT


def _get_nc(niter=1, kt=None):
    if kt is None:
        kt = _CUR_KT
    key = (niter, kt)
    if key not in _NC_CACHE:
        _NC_CACHE[key] = _build_bass(niter, kt)
    return _NC_CACHE[key]


_RUNNER_CACHE = {}


def _get_runner(kt):
    """Compile once, reuse across kernel() calls. Returns a callable
    taking concatenated input arrays and returning (out_u, sums) stacked
    per core."""
    if kt in _RUNNER_CACHE:
        return _RUNNER_CACHE[kt]

    import jax
    from jax.sharding import Mesh, PartitionSpec, NamedSharding
    from jax.experimental.shard_map import shard_map
    from concourse.bass2jax import (
        _bass_exec_p, install_neuronx_cc_hook, partition_id_tensor,
    )

    nc = _get_nc(1, kt)
    install_neuronx_cc_hook()
    in_names = []
    out_names = []
    out_avals = []
    zero_like = []
    part_name = nc.partition_id_tensor.name if nc.partition_id_tensor else None
    for alloc in nc.m.functions[0].allocations:
        if not isinstance(alloc, mybir.MemoryLocationSet):
            continue
        name = alloc.memorylocations[0].name
        if alloc.kind == "ExternalInput":
            if name != part_name:
                in_names.append(name)
        elif alloc.kind == "ExternalOutput":
            np_dt = mybir.dt.np(alloc.dtype)
            out_avals.append(jax.core.ShapedArray(tuple(alloc.tensor_shape), np_dt))
            out_names.append(name)
            zero_like.append((tuple(alloc.tensor_shape), np_dt))
    n_params = len(in_names)
    bind_in_names = tuple(in_names + out_names + ([part_name] if part_name else []))

    def _body(*args):
        ins = list(args[:n_params])
        outs = list(args[n_params:])
        extra = [partition_id_tensor()] if part_name else []
        outs = list(_bass_exec_p.bind(
            *ins, *outs, *extra,
            out_avals=tuple(out_avals),
            in_names=bind_in_names,
            out_names=tuple(out_names),
            lowering_input_output_aliases=(),
            sim_require_finite=True,
            sim_require_nnan=True,
            nc=nc,
        ))
        return tuple(outs)

    devices = jax.devices()[:NCORES]
    mesh = Mesh(np.asarray(devices), ("core",))
    n_outs = len(out_names)
    sharded = jax.jit(
        shard_map(
            _body, mesh=mesh,
            in_specs=(PartitionSpec("core"),) * (n_params + n_outs),
            out_specs=(PartitionSpec("core"),) * n_outs,
            check_rep=False,
        ),
        donate_argnums=tuple(range(n_params, n_params + n_outs)),
        keep_unused=True,
    )

    sh = NamedSharding(mesh, PartitionSpec("core"))
    import jax.numpy as jnp
    zeros_fn = jax.jit(
        lambda: tuple(
            jnp.zeros((NCORES * s[0],) + s[1:], d) for s, d in zero_like
        ),
        out_shardings=(sh,) * n_outs,
    )

    def run(per_core):
        # [8, s0, ...] -> [8*s0, ...] is a reshape view, not a copy
        concat_in = [
            np.ascontiguousarray(per_core[n]).reshape(
                (NCORES * per_core[n].shape[1],) + tuple(per_core[n].shape[2:])
            )
            for n in in_names
        ]
        # donated output buffers created on-device: avoids shipping 32 MB
        # of zeros over the (slow) axon link every call
        zeros = zeros_fn()
        outs = sharded(*concat_in, *zeros)
        res = {}
        for i, name in enumerate(out_names):
            a = np.asarray(outs[i])
            res[name] = a.reshape(NCORES, *out_avals[i].shape)
        return res

    _RUNNER_CACHE[kt] = run
    return run


def _pack_inputs(q, k, v, ratio, scale, attn_mask):
    """Host-side packing into the per-core flat layouts.

    Drops masked keys entirely (they contribute exactly 0 after exp) and
    packs the kept keys contiguously, padded to KT*128 with -1e9-bias
    slots. Sets the module-level _CUR_KT so a subsequent _get_nc() builds
    the matching kernel."""
    global _CUR_KT
    q = np.asarray(q, dtype=np.float32)
    k = np.asarray(k, dtype=np.float32)
    v = np.asarray(v, dtype=np.float32)
    ratio = np.asarray(ratio, dtype=np.float32)
    mask = np.asarray(attn_mask).astype(bool)

    keep = [np.nonzero(~mask[b])[0] for b in range(B)]
    nmax = max(len(ix) for ix in keep)
    KT = max(1, -(-nmax // P))
    if KT > LK // P:
        KT = LK // P
    _CUR_KT = KT
    KTP = KT * P

    mult = np.float32(scale) * ratio  # [B]
    qs = q * mult[:, None, None]      # [B, LQ, D]

    kc = np.zeros((B, KTP, D), dtype=np.float32)
    vc = np.zeros((B, KTP, D), dtype=np.float32)
    bias = np.full((B, KTP), NEG, dtype=np.float32)
    for b in range(B):
        n = len(keep[b])
        kc[b, :n] = k[b, keep[b]]
        vc[b, :n] = v[b, keep[b]]
        bias[b, :n] = 0.0

    # K region: [B, 128(d_in_tile), DT*KTP], col = d*KTP + key
    kd = np.ascontiguousarray(kc.transpose(0, 2, 1)).reshape(B, DT, P, KTP)
    kreg = np.ascontiguousarray(kd.transpose(0, 2, 1, 3)).reshape(B, P, DT * KTP)
    # Q region: col = d*LQ + q
    qd = np.ascontiguousarray(qs.transpose(0, 2, 1)).reshape(B, DT, P, LQ)
    qreg = np.ascontiguousarray(qd.transpose(0, 2, 1, 3)).reshape(B, P, DT * LQ)
    kq = np.concatenate([kreg, qreg], axis=2).astype(NPBF16)  # [B, 128, cols]

    vvl = vc.reshape(B, KT, P, D).transpose(0, 2, 1, 3)
    vvl = np.ascontiguousarray(vvl).reshape(B, P, KT * D).astype(NPBF16)

    # consts[b, p, t] = bias for key t*128+p
    consts = np.ascontiguousarray(
        bias.reshape(B, KT, P).transpose(0, 2, 1)
    )

    ones = np.ones((P, 1), dtype=NPBF16)
    return kq, vvl, consts, ones


def kernel(q, k, v, ratio, scale, attn_mask):
    """Full inputs in, full output out. Shards batch across 8 cores."""
    q = np.asarray(q)
    k = np.asarray(k)
    v = np.asarray(v)
    ratio = np.asarray(ratio)
    scale = np.asarray(scale)
    attn_mask = np.asarray(attn_mask)
    assert q.shape == (B, LQ, D) and k.shape == (B, LK, D)
    kq, vvl, consts, ones = _pack_inputs(q, k, v, ratio, scale, attn_mask)
    run = _get_runner(_CUR_KT)
    per_core = {
        "kq": kq, "vv": vvl, "consts": consts,
        "onesd": np.broadcast_to(ones, (B,) + ones.shape),
    }
    res = run(per_core)
    out_un = res["out_u"]                          # [B, LQ, D]
    ssum = res["sums"].reshape(B, LQ)
    out = out_un / ssum[:, :, None]
    return out.astype(np.float32)


# revision 38
# speedup vs baseline: 1.5855x; 1.1215x over previous
"""Sparse-attention Bass kernel for Trainium2 (8 NeuronCores).

Problem (per batch element b of 8):
    scores = (q @ k^T) * scale            [2048, 2048]
    scores = where(mask[k], -1e9, scores)
    scores = scores * ratio[b]
    attn   = softmax(scores, axis=-1)
    out    = attn @ v                      [2048, 512]

Sharding: batch dim (8) -> one NeuronCore each (SPMD, same NEFF).

Key compaction: masked keys (~10%) contribute exactly 0 to both the
softmax numerator and denominator (exp(-1e9) == 0 in f32), so the host
drops them and packs only the kept keys, padded to a multiple of 128
with -1e9-bias slots. 15 key tiles instead of 16 -> 6.25% less PE work.

Device layout ("S^T layout"): scores are computed transposed,
S^T[k, q] = K @ Q^T (keys on partitions, queries on the free dim), so
  - the pad-slot bias is a per-partition bias -> fused into the exp
    activation on the Scalar engine for free,
  - the AV matmul (contraction over keys) needs no transposes:
    lhsT = P^T tile [128k, 128q] (stationary), rhs = V [128k, 512d],
  - softmax denominators (sum over keys = partitions) come from a
    ones-vector matmul over DVE-accumulated partials.

All matmul operands are bf16 (inputs quantized on host, exp output
written as bf16 by the Scalar engine): same 1 row/cycle PE stream rate
as float32r, but weight loads are 2-4x faster (FWL), shrinking the
per-matmul weight-switch overhead. Accumulation stays fp32 in PSUM;
rowsum accumulation stays fp32 on DVE. End-to-end error ~0.4% rms,
well inside the 2e-2 gate.

Normalization (divide by rowsum) is done on the host: the device returns
the unnormalized O = exp(S) @ V plus the row sums.
The scale*ratio[b] factor is folded into q on the host.

Written in raw Bass (explicit engine programs + semaphores): the walrus
build in this container allows at most ONE semaphore wait per
instruction, which the Tile scheduler's auto-generated waits violate.
Standalone wait_ge instructions sidestep the limit.

Engine roles:
  sync   (SP) : input DMAs (one HWDGE ring, FIFO -> per-chunk sems)
  tensor (PE) : QK^T matmuls, rowsum matmuls, AV matmuls (bf16)
  vector (DVE): partial rowsum accumulation (fp32)
  scalar (ACT): exp (+pad bias) -> bf16, PSUM->SBUF copies, output DMAs
                (on ACT's own HWDGE ring so they don't queue behind
                the input DMAs)
"""

import sys

for _p in ("/opt/trn_rl_repo", "/opt/pypackages"):
    if _p not in sys.path:
        sys.path.append(_p)

import numpy as np
from contextlib import ExitStack

import concourse.bass as bass
from concourse import mybir

B, LQ, LK, D = 8, 2048, 2048, 512
P = 128
NCORES = 8
F32 = mybir.dt.float32
F32R = mybir.dt.float32r
BF16 = mybir.dt.bfloat16
NPBF16 = mybir.dt.np(BF16)
NEG = np.float32(-1e9)

DT = D // P        # 4 d-tiles (contraction for QK^T)
QBS = 512          # queries per PSUM block (free dim of S^T)
QB = LQ // QBS     # 4 query superblocks
QTPB = QBS // P    # 4 query tiles (of 128) per superblock

DEFAULT_KT = 15    # key tiles after compaction (keys padded to KT*128)


def _build_bass(niter=1, kt=DEFAULT_KT):
    KT = kt
    KTP = KT * P           # packed key count
    KOFF = DT * KTP        # start of the Q region in kq
    KQ_COLS = KOFF + DT * LQ

    nc = bass.Bass()

    consts = nc.dram_tensor("consts", [P, KT], F32, kind="ExternalInput")
    onesd = nc.dram_tensor("onesd", [P, 1], BF16, kind="ExternalInput")
    # kq packing (bf16): K region cols [0, KOFF): col = d*KTP + key
    #                    Q region cols [KOFF, ...): col = KOFF + d*LQ + q
    kq = nc.dram_tensor("kq", [P, KQ_COLS], BF16, kind="ExternalInput")
    # v: partition p = key kt*128+p, col = kt*D + d
    vv = nc.dram_tensor("vv", [P, KT * D], BF16, kind="ExternalInput")
    out_u = nc.dram_tensor("out_u", [LQ, D], F32, kind="ExternalOutput")
    sums = nc.dram_tensor("sums", [QB, QBS], F32, kind="ExternalOutput")

    EXP = mybir.ActivationFunctionType.Exp

    with ExitStack() as ctx:
        e = ctx.enter_context

        # SBUF
        sb_consts = e(nc.sbuf_tensor("sb_consts", [P, KT], F32))
        sb_ones = e(nc.sbuf_tensor("sb_ones", [P, 1], BF16))
        sb_kq = e(nc.sbuf_tensor("sb_kq", [P, KQ_COLS], BF16))
        sb_v = e(nc.sbuf_tensor("sb_v", [P, KT * D], BF16))
        # exp(S^T) tiles: [128k, 512q] per (qb parity, key tile)
        sb_pt = [
            [e(nc.sbuf_tensor(f"sb_pt{par}_{k}", [P, QBS], BF16)) for k in range(KT)]
            for par in range(2)
        ]
        sb_osb = [e(nc.sbuf_tensor(f"sb_osb{qt}", [P, D], F32)) for qt in range(QTPB)]
        sb_rs = [e(nc.sbuf_tensor(f"sb_rs{par}", [1, QBS], F32)) for par in range(2)]
        # per-partition partial sums of exp tiles (DVE, fp32), rounded to
        # bf16 once at the end so the ones-matmul runs at bf16 rate
        sb_acc = [e(nc.sbuf_tensor(f"sb_acc{par}", [P, QBS], F32)) for par in range(2)]
        sb_accb = [e(nc.sbuf_tensor(f"sb_accb{par}", [P, QBS], BF16)) for par in range(2)]
        sb_tmp = e(nc.sbuf_tensor("sb_tmp", [P, QBS], F32))

        # PSUM: 8 banks
        ps = [e(nc.psum_tensor(f"ps{i}", [P, QBS], F32)) for i in range(4)]
        po = [e(nc.psum_tensor(f"po{i}", [P, D], F32)) for i in range(2)]
        rs = [e(nc.psum_tensor(f"rs{i}", [P, QBS], F32)) for i in range(2)]

        # one semaphore per input DMA chunk: HWDGE DMAs on one ring may
        # complete out of order, so a shared counter can't identify which
        # transfer landed
        s_consts = e(nc.semaphore("s_consts"))
        s_ones = e(nc.semaphore("s_ones"))
        s_ab = e(nc.semaphore("s_ab"))
        s_c = [e(nc.semaphore(f"s_c{i}")) for i in range(3)]
        s_d = [e(nc.semaphore(f"s_d{i}")) for i in range(3)]
        s_v = [e(nc.semaphore(f"s_v{i}")) for i in range(4)]
        # per-output-buffer DMA-completion semaphores (buffer reuse gates)
        s_osb = [e(nc.semaphore(f"s_osb{qt}")) for qt in range(QTPB)]
        s_rsb = [e(nc.semaphore(f"s_rsb{par}")) for par in range(2)]
        pe_sem = e(nc.semaphore("pe_sem"))
        act_sem = e(nc.semaphore("act_sem"))
        dve_sem = e(nc.semaphore("dve_sem"))

        # ---- semaphore tick bookkeeping ----
        # gb = global block index (niter * QB blocks total); data block
        # qb = gb % QB.
        # pe_sem increments per gb: KT QK-group finals, 1 rowsum final,
        # 4 AV finals = KT + 5.
        # PE order per block: KT QK groups, AV qt0, rowsum MM, AV qt1-3.
        PEB = KT + 5
        # act_sem order per block: KT exps, rs copy, po0..po3 copies
        ACB = KT + 5

        def tick_qk(gb, k):
            return gb * PEB + k + 1

        def tick_av(gb, qt):
            return gb * PEB + (KT + 1 if qt == 0 else KT + 2 + qt)

        def tick_rs(gb):
            return gb * PEB + KT + 2

        def tick_acc(gb):
            # dve_sem: KT-1 accumulate-adds per block (KT >= 2)
            return max(KT - 1, 0) * (gb + 1)

        def tick_exp(gb, k):
            return gb * ACB + k + 1

        def tick_rsc(gb):
            return gb * ACB + KT + 1

        def tick_poc(gb, qt):
            return gb * ACB + KT + 2 + qt

        # K-chunk boundaries (in key-tile units) for the 3 "C" DMAs:
        # tiles 4..KT-1 split as evenly as possible into 3 chunks
        c_bounds = [4 + ((KT - 4) * i) // 3 for i in range(4)]  # e.g. [4,7,11,15]

        def c_idx_for_tile(k):  # which s_c gates key tile k (k >= 4)
            for i in range(3):
                if k < c_bounds[i + 1]:
                    return i
            return 2

        # V quarters (in key-tile units)
        v_bounds = [(KT * i) // 4 for i in range(5)]

        def v_idx_for_tile(k):
            for i in range(4):
                if k < v_bounds[i + 1]:
                    return i
            return 3

        with nc.Block() as block:

            @block.sync
            def _(sync):
                # issue order == consumption order so the PE rarely starves:
                # consts/ones, K(tiles 0-3), Q(block 0), K rest by range,
                # V by quarter, Q blocks 1-3
                sync.dma_start(sb_consts[:, :], consts[:, :]).then_inc(s_consts, 16)
                sync.dma_start(sb_ones[:, :], onesd[:, :]).then_inc(s_ones, 16)

                def k3d(t):  # view of the K region as [128, d=4, KTP]
                    return t[:, 0:KOFF].rearrange("p (d j) -> p d j", d=DT)

                def q3d(t):  # view of the Q region as [128, d=4, LQ]
                    return t[:, KOFF:KOFF + DT * LQ].rearrange(
                        "p (d j) -> p d j", d=DT
                    )

                sync.dma_start(
                    k3d(sb_kq)[:, :, 0:4 * P], k3d(kq)[:, :, 0:4 * P]
                ).then_inc(s_ab, 16)
                sync.dma_start(
                    q3d(sb_kq)[:, :, 0:QBS], q3d(kq)[:, :, 0:QBS]
                ).then_inc(s_ab, 16)
                for i in range(3):
                    if c_bounds[i] == c_bounds[i + 1]:
                        continue  # empty chunk (small KT); never waited on
                    js = slice(c_bounds[i] * P, c_bounds[i + 1] * P)
                    sync.dma_start(
                        k3d(sb_kq)[:, :, js], k3d(kq)[:, :, js]
                    ).then_inc(s_c[i], 16)
                for i in range(4):
                    if v_bounds[i] == v_bounds[i + 1]:
                        continue
                    cs = slice(v_bounds[i] * D, v_bounds[i + 1] * D)
                    sync.dma_start(sb_v[:, cs], vv[:, cs]).then_inc(s_v[i], 16)
                for i in range(3):
                    js = slice((i + 1) * QBS, (i + 2) * QBS)
                    sync.dma_start(
                        q3d(sb_kq)[:, :, js], q3d(kq)[:, :, js]
                    ).then_inc(s_d[i], 16)

            @block.tensor
            def _(tensor):
                last_wait = {}  # sem name -> value already waited for

                def wait(sem, val, name):
                    if val > last_wait.get(name, -1):
                        tensor.wait_ge(sem, val)
                        last_wait[name] = val

                for gb in range(niter * QB):
                    qb = gb % QB
                    # ---- QK^T phase ----
                    for k in range(KT):
                        g = gb * KT + k  # global k-iteration index
                        # input availability
                        if qb == 0:
                            if k < 4:
                                wait(s_ab, 32, "ab")
                            else:
                                ci = c_idx_for_tile(k)
                                wait(s_c[ci], 16, f"c{ci}")
                        else:
                            wait(s_d[qb - 1], 16, f"d{qb - 1}")
                        # ps[g%4] must have been consumed by exp of g-4.
                        # stride 2: waiting for exp(g-3) covers groups g and
                        # g+1 with one instruction, and exp(g-3) is ~2.5
                        # groups in the past so the wait never stalls
                        if g >= 4 and g % 2 == 0:
                            g3 = g - 3
                            wait(act_sem, tick_exp(g3 // KT, g3 % KT), "act")
                        for d in range(DT):
                            col = d * KTP + k * P
                            mm = tensor.matmul(
                                ps[g % 4][:, :],
                                lhsT=sb_kq[:, col:col + P],
                                rhs=sb_kq[
                                    :, KOFF + d * LQ + qb * QBS:
                                    KOFF + d * LQ + qb * QBS + QBS
                                ],
                                start=(d == 0),
                                stop=(d == DT - 1),
                            )
                            if d == DT - 1:
                                mm.then_inc(pe_sem, 1)

                    # ---- AV phase ----
                    for qt in range(QTPB):
                        # po[qt%2] consumed by copy of (gb,qt-2) / (gb-1,qt+2)
                        if qt >= 2:
                            wait(act_sem, tick_poc(gb, qt - 2), "act")
                        elif gb >= 1:
                            wait(act_sem, tick_poc(gb - 1, qt + 2), "act")
                        if qt == 0:
                            # exps 0..KT-3 are long done by now (ACT trails
                            # the QK phase by ~1 tile); one wait covers them
                            wait(act_sem, tick_exp(gb, max(KT - 3, 0)), "act")
                        for k in range(KT):
                            if qt == 0:
                                if k >= KT - 2:
                                    wait(act_sem, tick_exp(gb, k), "act")
                                vi = v_idx_for_tile(k)
                                wait(s_v[vi], 16, f"v{vi}")
                            mm = tensor.matmul(
                                po[qt % 2][:, :],
                                lhsT=sb_pt[gb % 2][k][:, qt * P:(qt + 1) * P],
                                rhs=sb_v[:, k * D:(k + 1) * D],
                                start=(k == 0),
                                stop=(k == KT - 1),
                            )
                            if k == KT - 1:
                                mm.then_inc(pe_sem, 1)
                        if qt == 0:
                            # single partition-reduction matmul over the
                            # DVE-accumulated exp sums
                            wait(s_ones, 16, "ones")
                            if KT > 1:
                                wait(dve_sem, tick_acc(gb), "dve")
                                rs_rhs = sb_accb[gb % 2][:, :]
                            else:
                                rs_rhs = sb_pt[gb % 2][0][:, :]
                            tensor.matmul(
                                rs[gb % 2][0:1, :],
                                lhsT=sb_ones[:, :],
                                rhs=rs_rhs,
                                start=True,
                                stop=True,
                            ).then_inc(pe_sem, 1)

            @block.vector
            def _(vector):
                last_wait = {}

                def wait(sem, val, name):
                    if val > last_wait.get(name, -1):
                        vector.wait_ge(sem, val)
                        last_wait[name] = val

                ndve = 0
                for gb in range(niter * QB):
                    # accb[gb%2] readable again after PE's rowsum MM of gb-2
                    if gb >= 2:
                        wait(pe_sem, tick_rs(gb - 2), "pe")
                    par = gb % 2
                    if KT == 2:
                        wait(act_sem, tick_exp(gb, 1), "act")
                        vector.tensor_add(
                            sb_accb[par][:, :],
                            sb_pt[par][0][:, :], sb_pt[par][1][:, :],
                        ).then_inc(dve_sem, 1)
                        ndve += 1
                    elif KT == 3:
                        wait(act_sem, tick_exp(gb, 1), "act")
                        vector.tensor_add(
                            sb_acc[par][:, :],
                            sb_pt[par][0][:, :], sb_pt[par][1][:, :],
                        ).then_inc(dve_sem, 1)
                        ndve += 1
                        wait(act_sem, tick_exp(gb, 2), "act")
                        wait(dve_sem, ndve, "dve")
                        vector.tensor_add(
                            sb_accb[par][:, :],
                            sb_acc[par][:, :], sb_pt[par][2][:, :],
                        ).then_inc(dve_sem, 1)
                        ndve += 1
                    elif KT >= 4:
                        # fp32 accumulation of pt0..pt[KT-3] into acc, the
                        # last two tiles into tmp, then a single bf16
                        # rounding: accb = acc + tmp
                        for j in range(1, KT - 2):
                            wait(act_sem, tick_exp(gb, j), "act")
                            if j > 1:
                                # same-engine RAW on acc: wait for own pipe
                                # drain
                                wait(dve_sem, ndve, "dve")
                            vector.tensor_add(
                                sb_acc[par][:, :],
                                sb_pt[par][0][:, :] if j == 1
                                else sb_acc[par][:, :],
                                sb_pt[par][j][:, :],
                            ).then_inc(dve_sem, 1)
                            ndve += 1
                        wait(act_sem, tick_exp(gb, KT - 1), "act")
                        vector.tensor_add(
                            sb_tmp[:, :],
                            sb_pt[par][KT - 2][:, :],
                            sb_pt[par][KT - 1][:, :],
                        ).then_inc(dve_sem, 1)
                        ndve += 1
                        wait(dve_sem, ndve, "dve")
                        vector.tensor_add(
                            sb_accb[par][:, :],
                            sb_acc[par][:, :], sb_tmp[:, :],
                        ).then_inc(dve_sem, 1)
                        ndve += 1

            @block.scalar
            def _(scalar):
                last_wait = {}

                def wait(sem, val, name):
                    if val > last_wait.get(name, -1):
                        scalar.wait_ge(sem, val)
                        last_wait[name] = val

                wait(s_consts, 16, "consts")
                for gb in range(niter * QB):
                    qb = gb % QB
                    for k in range(KT):
                        g = gb * KT + k
                        wait(pe_sem, tick_qk(gb, k), "pe")
                        scalar.activation(
                            sb_pt[gb % 2][k][:, :],
                            ps[g % 4][:, :],
                            EXP,
                            bias=sb_consts[:, k:k + 1],
                            scale=1.0,
                        ).then_inc(act_sem, 1)
                    # rowsum copy + DMA (ACT's own HWDGE ring)
                    if gb >= 2:
                        wait(s_rsb[gb % 2], 16 * (gb // 2), f"rsb{gb % 2}")
                    wait(pe_sem, tick_rs(gb), "pe")
                    scalar.copy(sb_rs[gb % 2][:, :], rs[gb % 2][0:1, :]).then_inc(
                        act_sem, 1
                    )
                    # self-wait: the DMA engine reads sb_rs asynchronously,
                    # so the copy must have fully drained first
                    wait(act_sem, tick_rsc(gb), "act")
                    scalar.dma_start(
                        sums[qb:qb + 1, :], sb_rs[gb % 2][:, :]
                    ).then_inc(s_rsb[gb % 2], 16)
                    # AV output copies + DMAs
                    for qt in range(QTPB):
                        if gb >= 1:
                            wait(s_osb[qt], 16 * gb, f"osb{qt}")
                        wait(pe_sem, tick_av(gb, qt), "pe")
                        scalar.copy(sb_osb[qt][:, :], po[qt % 2][:, :]).then_inc(
                            act_sem, 1
                        )
                        wait(act_sem, tick_poc(gb, qt), "act")
                        row = (qb * QTPB + qt) * P
                        scalar.dma_start(
                            out_u[row:row + P, :], sb_osb[qt][:, :]
                        ).then_inc(s_osb[qt], 16)
                # drain: all output DMAs landed
                for qt in range(QTPB):
                    scalar.wait_ge(s_osb[qt], 16 * QB * niter)
                for par in range(2):
                    scalar.wait_ge(s_rsb[par], 16 * 2 * niter)

    return nc


_NC_CACHE = {}

# KT used by the most recent _pack_inputs call; _get_nc defaults to it so
# the pack -> compile -> run sequence stays consistent.
_CUR_KT = DEFAULT_KT


def _get_nc(niter=1, kt=None):
    if kt is None:
        kt = _CUR_KT
    key = (niter, kt)
    if key not in _NC_CACHE:
        _NC_CACHE[key] = _build_bass(niter, kt)
    return _NC_CACHE[key]


_RUNNER_CACHE = {}


def _get_runner(kt):
    """Compile once, reuse across kernel() calls. Returns a callable
    taking concatenated input arrays and returning (out_u, sums) stacked
    per core."""
    if kt in _RUNNER_CACHE:
        return _RUNNER_CACHE[kt]

    import jax
    from jax.sharding import Mesh, PartitionSpec, NamedSharding
    from jax.experimental.shard_map import shard_map
    from concourse.bass2jax import (
        _bass_exec_p, install_neuronx_cc_hook, partition_id_tensor,
    )

    nc = _get_nc(1, kt)
    install_neuronx_cc_hook()
    in_names = []
    out_names = []
    out_avals = []
    zero_like = []
    part_name = nc.partition_id_tensor.name if nc.partition_id_tensor else None
    for alloc in nc.m.functions[0].allocations:
        if not isinstance(alloc, mybir.MemoryLocationSet):
            continue
        name = alloc.memorylocations[0].name
        if alloc.kind == "ExternalInput":
            if name != part_name:
                in_names.append(name)
        elif alloc.kind == "ExternalOutput":
            np_dt = mybir.dt.np(alloc.dtype)
            out_avals.append(jax.core.ShapedArray(tuple(alloc.tensor_shape), np_dt))
            out_names.append(name)
            zero_like.append((tuple(alloc.tensor_shape), np_dt))
    n_params = len(in_names)
    bind_in_names = tuple(in_names + out_names + ([part_name] if part_name else []))

    def _body(*args):
        ins = list(args[:n_params])
        outs = list(args[n_params:])
        extra = [partition_id_tensor()] if part_name else []
        outs = list(_bass_exec_p.bind(
            *ins, *outs, *extra,
            out_avals=tuple(out_avals),
            in_names=bind_in_names,
            out_names=tuple(out_names),
            lowering_input_output_aliases=(),
            sim_require_finite=True,
            sim_require_nnan=True,
            nc=nc,
        ))
        return tuple(outs)

    devices = jax.devices()[:NCORES]
    mesh = Mesh(np.asarray(devices), ("core",))
    n_outs = len(out_names)
    sharded = jax.jit(
        shard_map(
            _body, mesh=mesh,
            in_specs=(PartitionSpec("core"),) * (n_params + n_outs),
            out_specs=(PartitionSpec("core"),) * n_outs,
            check_rep=False,
        ),
        donate_argnums=tuple(range(n_params, n_params + n_outs)),
        keep_unused=True,
    )

    sh = NamedSharding(mesh, PartitionSpec("core"))
    import jax.numpy as jnp
    zeros_fn = jax.jit(
        lambda: tuple(
            jnp.zeros((NCORES * s[0],) + s[1:], d) for s, d in zero_like
        ),
        out_shardings=(sh,) * n_outs,
    )

    def run(per_core):
        # [8, s0, ...] -> [8*s0, ...] is a reshape view, not a copy
        concat_in = [
            np.ascontiguousarray(per_core[n]).reshape(
                (NCORES * per_core[n].shape[1],) + tuple(per_core[n].shape[2:])
            )
            for n in in_names
        ]
        # donated output buffers created on-device: avoids shipping 32 MB
        # of zeros over the (slow) axon link every call
        zeros = zeros_fn()
        outs = sharded(*concat_in, *zeros)
        res = {}
        for i, name in enumerate(out_names):
            a = np.asarray(outs[i])
            res[name] = a.reshape(NCORES, *out_avals[i].shape)
        return res

    _RUNNER_CACHE[kt] = run
    return run


def _pack_inputs(q, k, v, ratio, scale, attn_mask):
    """Host-side packing into the per-core flat layouts.

    Drops masked keys entirely (they contribute exactly 0 after exp) and
    packs the kept keys contiguously, padded to KT*128 with -1e9-bias
    slots. Sets the module-level _CUR_KT so a subsequent _get_nc() builds
    the matching kernel."""
    global _CUR_KT
    q = np.asarray(q, dtype=np.float32)
    k = np.asarray(k, dtype=np.float32)
    v = np.asarray(v, dtype=np.float32)
    ratio = np.asarray(ratio, dtype=np.float32)
    mask = np.asarray(attn_mask).astype(bool)

    keep = [np.nonzero(~mask[b])[0] for b in range(B)]
    nmax = max(len(ix) for ix in keep)
    KT = max(1, -(-nmax // P))
    if KT > LK // P:
        KT = LK // P
    _CUR_KT = KT
    KTP = KT * P

    mult = np.float32(scale) * ratio  # [B]
    qs = q * mult[:, None, None]      # [B, LQ, D]

    kc = np.zeros((B, KTP, D), dtype=np.float32)
    vc = np.zeros((B, KTP, D), dtype=np.float32)
    bias = np.full((B, KTP), NEG, dtype=np.float32)
    for b in range(B):
        n = len(keep[b])
        kc[b, :n] = k[b, keep[b]]
        vc[b, :n] = v[b, keep[b]]
        bias[b, :n] = 0.0

    # K region: [B, 128(d_in_tile), DT*KTP], col = d*KTP + key
    kd = np.ascontiguousarray(kc.transpose(0, 2, 1)).reshape(B, DT, P, KTP)
    kreg = np.ascontiguousarray(kd.transpose(0, 2, 1, 3)).reshape(B, P, DT * KTP)
    # Q region: col = d*LQ + q
    qd = np.ascontiguousarray(qs.transpose(0, 2, 1)).reshape(B, DT, P, LQ)
    qreg = np.ascontiguousarray(qd.transpose(0, 2, 1, 3)).reshape(B, P, DT * LQ)
    kq = np.concatenate([kreg, qreg], axis=2).astype(NPBF16)  # [B, 128, cols]

    vvl = vc.reshape(B, KT, P, D).transpose(0, 2, 1, 3)
    vvl = np.ascontiguousarray(vvl).reshape(B, P, KT * D).astype(NPBF16)

    # consts[b, p, t] = bias for key t*128+p
    consts = np.ascontiguousarray(
        bias.reshape(B, KT, P).transpose(0, 2, 1)
    )

    ones = np.ones((P, 1), dtype=NPBF16)
    return kq, vvl, consts, ones


def kernel(q, k, v, ratio, scale, attn_mask):
    """Full inputs in, full output out. Shards batch across 8 cores."""
    q = np.asarray(q)
    k = np.asarray(k)
    v = np.asarray(v)
    ratio = np.asarray(ratio)
    scale = np.asarray(scale)
    attn_mask = np.asarray(attn_mask)
    assert q.shape == (B, LQ, D) and k.shape == (B, LK, D)
    kq, vvl, consts, ones = _pack_inputs(q, k, v, ratio, scale, attn_mask)
    run = _get_runner(_CUR_KT)
    per_core = {
        "kq": kq, "vv": vvl, "consts": consts,
        "onesd": np.broadcast_to(ones, (B,) + ones.shape),
    }
    res = run(per_core)
    out_un = res["out_u"]                          # [B, LQ, D]
    ssum = res["sums"].reshape(B, LQ)
    out = out_un / ssum[:, :, None]
    return out.astype(np.float32)


# revision 42
# speedup vs baseline: 1.8387x; 1.1597x over previous
"""Sparse-attention Bass kernel for Trainium2 (8 NeuronCores).

Problem (per batch element b of 8):
    scores = (q @ k^T) * scale            [2048, 2048]
    scores = where(mask[k], -1e9, scores)
    scores = scores * ratio[b]
    attn   = softmax(scores, axis=-1)
    out    = attn @ v                      [2048, 512]

Sharding: batch dim (8) -> one NeuronCore each (SPMD, same NEFF).

Key compaction: masked keys (~10%) contribute exactly 0 to both the
softmax numerator and denominator (exp(-1e9) == 0 in f32), so the host
drops them and packs only the kept keys, padded to a multiple of 128
with -1e9-bias slots. 15 key tiles instead of 16 -> 6.25% less PE work.

Device layout ("S^T layout"): scores are computed transposed,
S^T[k, q] = K @ Q^T (keys on partitions, queries on the free dim), so
  - the pad-slot bias is a per-partition bias -> fused into the exp
    activation on the Scalar engine for free,
  - the AV matmul (contraction over keys) needs no transposes:
    lhsT = P^T tile [128k, 128q] (stationary), rhs = V [128k, 512d],
  - softmax denominators (sum over keys = partitions) come from a
    ones-vector matmul over DVE-accumulated partials.

All matmul operands are bf16 (inputs quantized on host, exp output
written as bf16 by the Scalar engine): same 1 row/cycle PE stream rate
as float32r, but weight loads are 2-4x faster (FWL), shrinking the
per-matmul weight-switch overhead. Accumulation stays fp32 in PSUM;
rowsum accumulation stays fp32 on DVE. End-to-end error ~0.4% rms,
well inside the 2e-2 gate.

Normalization (divide by rowsum) is done on the host: the device returns
the unnormalized O = exp(S) @ V plus the row sums.
The scale*ratio[b] factor is folded into q on the host.

Written in raw Bass (explicit engine programs + semaphores): the walrus
build in this container allows at most ONE semaphore wait per
instruction, which the Tile scheduler's auto-generated waits violate.
Standalone wait_ge instructions sidestep the limit.

Engine roles:
  sync   (SP) : input DMAs (one HWDGE ring, FIFO -> per-chunk sems)
  tensor (PE) : QK^T matmuls, rowsum matmuls, AV matmuls (bf16)
  vector (DVE): partial rowsum accumulation (fp32)
  scalar (ACT): exp (+pad bias) -> bf16, PSUM->SBUF copies, output DMAs
                (on ACT's own HWDGE ring so they don't queue behind
                the input DMAs)
"""

import sys

for _p in ("/opt/trn_rl_repo", "/opt/pypackages"):
    if _p not in sys.path:
        sys.path.append(_p)

import numpy as np
from contextlib import ExitStack

import concourse.bass as bass
from concourse import mybir

B, LQ, LK, D = 8, 2048, 2048, 512
P = 128
NCORES = 8
F32 = mybir.dt.float32
F32R = mybir.dt.float32r
BF16 = mybir.dt.bfloat16
NPBF16 = mybir.dt.np(BF16)
NEG = np.float32(-1e9)

DT = D // P        # 4 d-tiles (contraction for QK^T)
QBS = 512          # queries per PSUM block (free dim of S^T)
QB = LQ // QBS     # 4 query superblocks
QTPB = QBS // P    # 4 query tiles (of 128) per superblock

DEFAULT_KT = 15    # key tiles after compaction (keys padded to KT*128)


def _build_bass(niter=1, kt=DEFAULT_KT):
    KT = kt
    KTP = KT * P           # packed key count
    KOFF = DT * KTP        # start of the Q region in kq
    KQ_COLS = KOFF + DT * LQ

    nc = bass.Bass()

    consts = nc.dram_tensor("consts", [P, KT], F32, kind="ExternalInput")
    # full 128x128 ones matrix: a 1-col stationary lowers to a col_grp-
    # masked LDWEIGHTS that the PE reorder window cannot pull ahead of
    # in-flight matmuls (measured ~+93 ns twice per block); a full-width
    # stationary prefetches like every other weight load
    onesd = nc.dram_tensor("onesd", [P, P], BF16, kind="ExternalInput")
    # kq packing (bf16): K region cols [0, KOFF): col = d*KTP + key
    #                    Q region cols [KOFF, ...): col = KOFF + d*LQ + q
    kq = nc.dram_tensor("kq", [P, KQ_COLS], BF16, kind="ExternalInput")
    # v: partition p = key kt*128+p, col = kt*D + d
    vv = nc.dram_tensor("vv", [P, KT * D], BF16, kind="ExternalInput")
    out_u = nc.dram_tensor("out_u", [LQ, D], F32, kind="ExternalOutput")
    sums = nc.dram_tensor("sums", [QB, QBS], F32, kind="ExternalOutput")

    EXP = mybir.ActivationFunctionType.Exp

    with ExitStack() as ctx:
        e = ctx.enter_context

        # SBUF
        sb_consts = e(nc.sbuf_tensor("sb_consts", [P, KT], F32))
        sb_ones = e(nc.sbuf_tensor("sb_ones", [P, P], BF16))
        sb_kq = e(nc.sbuf_tensor("sb_kq", [P, KQ_COLS], BF16))
        sb_v = e(nc.sbuf_tensor("sb_v", [P, KT * D], BF16))
        # exp(S^T) tiles: [128k, 512q] per (qb parity, key tile)
        sb_pt = [
            [e(nc.sbuf_tensor(f"sb_pt{par}_{k}", [P, QBS], BF16)) for k in range(KT)]
            for par in range(2)
        ]
        sb_osb = [e(nc.sbuf_tensor(f"sb_osb{qt}", [P, D], F32)) for qt in range(QTPB)]
        sb_rs = [e(nc.sbuf_tensor(f"sb_rs{par}", [1, QBS], F32)) for par in range(2)]
        # per-partition partial sums of exp tiles (DVE, fp32), rounded to
        # bf16 once at the end so the ones-matmul runs at bf16 rate
        sb_acc = [e(nc.sbuf_tensor(f"sb_acc{par}", [P, QBS], F32)) for par in range(2)]
        sb_accb = [e(nc.sbuf_tensor(f"sb_accb{par}", [P, QBS], BF16)) for par in range(2)]
        sb_tmp = e(nc.sbuf_tensor("sb_tmp", [P, QBS], F32))

        # PSUM: 8 banks
        ps = [e(nc.psum_tensor(f"ps{i}", [P, QBS], F32)) for i in range(4)]
        po = [e(nc.psum_tensor(f"po{i}", [P, D], F32)) for i in range(2)]
        rs = [e(nc.psum_tensor(f"rs{i}", [P, QBS], F32)) for i in range(2)]

        # one semaphore per input DMA chunk: HWDGE DMAs on one ring may
        # complete out of order, so a shared counter can't identify which
        # transfer landed
        s_consts = e(nc.semaphore("s_consts"))
        s_ones = e(nc.semaphore("s_ones"))
        s_ab = e(nc.semaphore("s_ab"))
        s_c = [e(nc.semaphore(f"s_c{i}")) for i in range(3)]
        s_d = [e(nc.semaphore(f"s_d{i}")) for i in range(3)]
        s_v = [e(nc.semaphore(f"s_v{i}")) for i in range(4)]
        # per-output-buffer DMA-completion semaphores (buffer reuse gates)
        s_osb = [e(nc.semaphore(f"s_osb{qt}")) for qt in range(QTPB)]
        s_rsb = [e(nc.semaphore(f"s_rsb{par}")) for par in range(2)]
        pe_sem = e(nc.semaphore("pe_sem"))
        act_sem = e(nc.semaphore("act_sem"))
        dve_sem = e(nc.semaphore("dve_sem"))

        # ---- semaphore tick bookkeeping ----
        # gb = global block index (niter * QB blocks total); data block
        # qb = gb % QB.
        # pe_sem increments per gb: KT QK-group finals, 1 rowsum final,
        # 4 AV finals = KT + 5.
        # PE order per block: KT QK groups, AV qt0, rowsum MM, AV qt1-3.
        PEB = KT + 5
        # act_sem order per block: KT exps, rs copy, po0..po3 copies
        ACB = KT + 5

        def tick_qk(gb, k):
            return gb * PEB + k + 1

        def tick_av(gb, qt):
            return gb * PEB + (KT + 1 if qt == 0 else KT + 2 + qt)

        def tick_rs(gb):
            return gb * PEB + KT + 2

        def tick_acc(gb):
            # dve_sem: KT-1 accumulate-adds per block (KT >= 2)
            return max(KT - 1, 0) * (gb + 1)

        def tick_exp(gb, k):
            return gb * ACB + k + 1

        def tick_rsc(gb):
            return gb * ACB + KT + 1

        def tick_poc(gb, qt):
            return gb * ACB + KT + 2 + qt

        # K-chunk boundaries (in key-tile units) for the 3 "C" DMAs:
        # tiles 4..KT-1 split as evenly as possible into 3 chunks
        c_bounds = [4 + ((KT - 4) * i) // 3 for i in range(4)]  # e.g. [4,7,11,15]

        def c_idx_for_tile(k):  # which s_c gates key tile k (k >= 4)
            for i in range(3):
                if k < c_bounds[i + 1]:
                    return i
            return 2

        # V quarters (in key-tile units)
        v_bounds = [(KT * i) // 4 for i in range(5)]

        def v_idx_for_tile(k):
            for i in range(4):
                if k < v_bounds[i + 1]:
                    return i
            return 3

        with nc.Block() as block:

            @block.sync
            def _(sync):
                # issue order == consumption order so the PE rarely starves:
                # consts/ones, K(tiles 0-3), Q(block 0), K rest by range,
                # V by quarter, Q blocks 1-3
                sync.dma_start(sb_consts[:, :], consts[:, :]).then_inc(s_consts, 16)
                sync.dma_start(sb_ones[:, :], onesd[:, :]).then_inc(s_ones, 16)

                def k3d(t):  # view of the K region as [128, d=4, KTP]
                    return t[:, 0:KOFF].rearrange("p (d j) -> p d j", d=DT)

                def q3d(t):  # view of the Q region as [128, d=4, LQ]
                    return t[:, KOFF:KOFF + DT * LQ].rearrange(
                        "p (d j) -> p d j", d=DT
                    )

                sync.dma_start(
                    k3d(sb_kq)[:, :, 0:4 * P], k3d(kq)[:, :, 0:4 * P]
                ).then_inc(s_ab, 16)
                sync.dma_start(
                    q3d(sb_kq)[:, :, 0:QBS], q3d(kq)[:, :, 0:QBS]
                ).then_inc(s_ab, 16)
                for i in range(3):
                    if c_bounds[i] == c_bounds[i + 1]:
                        continue  # empty chunk (small KT); never waited on
                    js = slice(c_bounds[i] * P, c_bounds[i + 1] * P)
                    sync.dma_start(
                        k3d(sb_kq)[:, :, js], k3d(kq)[:, :, js]
                    ).then_inc(s_c[i], 16)
                for i in range(4):
                    if v_bounds[i] == v_bounds[i + 1]:
                        continue
                    cs = slice(v_bounds[i] * D, v_bounds[i + 1] * D)
                    sync.dma_start(sb_v[:, cs], vv[:, cs]).then_inc(s_v[i], 16)
                for i in range(3):
                    js = slice((i + 1) * QBS, (i + 2) * QBS)
                    sync.dma_start(
                        q3d(sb_kq)[:, :, js], q3d(kq)[:, :, js]
                    ).then_inc(s_d[i], 16)

            @block.tensor
            def _(tensor):
                last_wait = {}  # sem name -> value already waited for

                def wait(sem, val, name):
                    if val > last_wait.get(name, -1):
                        tensor.wait_ge(sem, val)
                        last_wait[name] = val

                for gb in range(niter * QB):
                    qb = gb % QB
                    # ---- QK^T phase ----
                    for k in range(KT):
                        g = gb * KT + k  # global k-iteration index
                        # input availability
                        if qb == 0:
                            if k < 4:
                                wait(s_ab, 32, "ab")
                            else:
                                ci = c_idx_for_tile(k)
                                wait(s_c[ci], 16, f"c{ci}")
                        else:
                            wait(s_d[qb - 1], 16, f"d{qb - 1}")
                        # ps[g%4] must have been consumed by exp of g-4.
                        # stride 2: waiting for exp(g-3) covers groups g and
                        # g+1 with one instruction, and exp(g-3) is ~2.5
                        # groups in the past so the wait never stalls
                        if g >= 4 and g % 2 == 0:
                            g3 = g - 3
                            wait(act_sem, tick_exp(g3 // KT, g3 % KT), "act")
                        for d in range(DT):
                            col = d * KTP + k * P
                            mm = tensor.matmul(
                                ps[g % 4][:, :],
                                lhsT=sb_kq[:, col:col + P],
                                rhs=sb_kq[
                                    :, KOFF + d * LQ + qb * QBS:
                                    KOFF + d * LQ + qb * QBS + QBS
                                ],
                                start=(d == 0),
                                stop=(d == DT - 1),
                            )
                            if d == DT - 1:
                                mm.then_inc(pe_sem, 1)

                    # ---- AV phase ----
                    for qt in range(QTPB):
                        # po[qt%2] consumed by copy of (gb,qt-2) / (gb-1,qt+2)
                        if qt >= 2:
                            wait(act_sem, tick_poc(gb, qt - 2), "act")
                        elif gb >= 1:
                            wait(act_sem, tick_poc(gb - 1, qt + 2), "act")
                        if qt == 0:
                            # exps 0..KT-3 are long done by now (ACT trails
                            # the QK phase by ~1 tile); one wait covers them
                            wait(act_sem, tick_exp(gb, max(KT - 3, 0)), "act")
                        for k in range(KT):
                            if qt == 0:
                                if k >= KT - 2:
                                    wait(act_sem, tick_exp(gb, k), "act")
                                vi = v_idx_for_tile(k)
                                wait(s_v[vi], 16, f"v{vi}")
                            mm = tensor.matmul(
                                po[qt % 2][:, :],
                                lhsT=sb_pt[gb % 2][k][:, qt * P:(qt + 1) * P],
                                rhs=sb_v[:, k * D:(k + 1) * D],
                                start=(k == 0),
                                stop=(k == KT - 1),
                            )
                            if k == KT - 1:
                                mm.then_inc(pe_sem, 1)
                        if qt == 0:
                            # single partition-reduction matmul over the
                            # DVE-accumulated exp sums
                            wait(s_ones, 16, "ones")
                            if KT > 1:
                                wait(dve_sem, tick_acc(gb), "dve")
                                rs_rhs = sb_accb[gb % 2][:, :]
                            else:
                                rs_rhs = sb_pt[gb % 2][0][:, :]
                            tensor.matmul(
                                rs[gb % 2][:, :],
                                lhsT=sb_ones[:, :],
                                rhs=rs_rhs,
                                start=True,
                                stop=True,
                            ).then_inc(pe_sem, 1)

            @block.vector
            def _(vector):
                last_wait = {}

                def wait(sem, val, name):
                    if val > last_wait.get(name, -1):
                        vector.wait_ge(sem, val)
                        last_wait[name] = val

                ndve = 0
                for gb in range(niter * QB):
                    # accb[gb%2] readable again after PE's rowsum MM of gb-2
                    if gb >= 2:
                        wait(pe_sem, tick_rs(gb - 2), "pe")
                    par = gb % 2
                    if KT == 2:
                        wait(act_sem, tick_exp(gb, 1), "act")
                        vector.tensor_add(
                            sb_accb[par][:, :],
                            sb_pt[par][0][:, :], sb_pt[par][1][:, :],
                        ).then_inc(dve_sem, 1)
                        ndve += 1
                    elif KT == 3:
                        wait(act_sem, tick_exp(gb, 1), "act")
                        vector.tensor_add(
                            sb_acc[par][:, :],
                            sb_pt[par][0][:, :], sb_pt[par][1][:, :],
                        ).then_inc(dve_sem, 1)
                        ndve += 1
                        wait(act_sem, tick_exp(gb, 2), "act")
                        wait(dve_sem, ndve, "dve")
                        vector.tensor_add(
                            sb_accb[par][:, :],
                            sb_acc[par][:, :], sb_pt[par][2][:, :],
                        ).then_inc(dve_sem, 1)
                        ndve += 1
                    elif KT >= 4:
                        # fp32 accumulation of pt0..pt[KT-3] into acc, the
                        # last two tiles into tmp, then a single bf16
                        # rounding: accb = acc + tmp
                        for j in range(1, KT - 2):
                            wait(act_sem, tick_exp(gb, j), "act")
                            if j > 1:
                                # same-engine RAW on acc: wait for own pipe
                                # drain
                                wait(dve_sem, ndve, "dve")
                            vector.tensor_add(
                                sb_acc[par][:, :],
                                sb_pt[par][0][:, :] if j == 1
                                else sb_acc[par][:, :],
                                sb_pt[par][j][:, :],
                            ).then_inc(dve_sem, 1)
                            ndve += 1
                        wait(act_sem, tick_exp(gb, KT - 1), "act")
                        vector.tensor_add(
                            sb_tmp[:, :],
                            sb_pt[par][KT - 2][:, :],
                            sb_pt[par][KT - 1][:, :],
                        ).then_inc(dve_sem, 1)
                        ndve += 1
                        wait(dve_sem, ndve, "dve")
                        vector.tensor_add(
                            sb_accb[par][:, :],
                            sb_acc[par][:, :], sb_tmp[:, :],
                        ).then_inc(dve_sem, 1)
                        ndve += 1

            @block.scalar
            def _(scalar):
                last_wait = {}

                def wait(sem, val, name):
                    if val > last_wait.get(name, -1):
                        scalar.wait_ge(sem, val)
                        last_wait[name] = val

                wait(s_consts, 16, "consts")
                for gb in range(niter * QB):
                    qb = gb % QB
                    for k in range(KT):
                        g = gb * KT + k
                        wait(pe_sem, tick_qk(gb, k), "pe")
                        scalar.activation(
                            sb_pt[gb % 2][k][:, :],
                            ps[g % 4][:, :],
                            EXP,
                            bias=sb_consts[:, k:k + 1],
                            scale=1.0,
                        ).then_inc(act_sem, 1)
                    # rowsum copy + DMA (ACT's own HWDGE ring)
                    if gb >= 2:
                        wait(s_rsb[gb % 2], 16 * (gb // 2), f"rsb{gb % 2}")
                    wait(pe_sem, tick_rs(gb), "pe")
                    scalar.copy(sb_rs[gb % 2][:, :], rs[gb % 2][0:1, :]).then_inc(
                        act_sem, 1
                    )
                    # self-wait: the DMA engine reads sb_rs asynchronously,
                    # so the copy must have fully drained first
                    wait(act_sem, tick_rsc(gb), "act")
                    scalar.dma_start(
                        sums[qb:qb + 1, :], sb_rs[gb % 2][:, :]
                    ).then_inc(s_rsb[gb % 2], 16)
                    # AV output copies + DMAs
                    for qt in range(QTPB):
                        if gb >= 1:
                            wait(s_osb[qt], 16 * gb, f"osb{qt}")
                        wait(pe_sem, tick_av(gb, qt), "pe")
                        scalar.copy(sb_osb[qt][:, :], po[qt % 2][:, :]).then_inc(
                            act_sem, 1
                        )
                        wait(act_sem, tick_poc(gb, qt), "act")
                        row = (qb * QTPB + qt) * P
                        scalar.dma_start(
                            out_u[row:row + P, :], sb_osb[qt][:, :]
                        ).then_inc(s_osb[qt], 16)
                # drain: all output DMAs landed
                for qt in range(QTPB):
                    scalar.wait_ge(s_osb[qt], 16 * QB * niter)
                for par in range(2):
                    scalar.wait_ge(s_rsb[par], 16 * 2 * niter)

    return nc


_NC_CACHE = {}

# KT used by the most recent _pack_inputs call; _get_nc defaults to it so
# the pack -> compile -> run sequence stays consistent.
_CUR_KT = DEFAULT_KT


def _get_nc(niter=1, kt=None):
    if kt is None:
        kt = _CUR_KT
    key = (niter, kt)
    if key not in _NC_CACHE:
        _NC_CACHE[key] = _build_bass(niter, kt)
    return _NC_CACHE[key]


_RUNNER_CACHE = {}


def _get_runner(kt):
    """Compile once, reuse across kernel() calls. Returns a callable
    taking concatenated input arrays and returning (out_u, sums) stacked
    per core."""
    if kt in _RUNNER_CACHE:
        return _RUNNER_CACHE[kt]

    import jax
    from jax.sharding import Mesh, PartitionSpec, NamedSharding
    from jax.experimental.shard_map import shard_map
    from concourse.bass2jax import (
        _bass_exec_p, install_neuronx_cc_hook, partition_id_tensor,
    )

    nc = _get_nc(1, kt)
    install_neuronx_cc_hook()
    in_names = []
    out_names = []
    out_avals = []
    zero_like = []
    part_name = nc.partition_id_tensor.name if nc.partition_id_tensor else None
    for alloc in nc.m.functions[0].allocations:
        if not isinstance(alloc, mybir.MemoryLocationSet):
            continue
        name = alloc.memorylocations[0].name
        if alloc.kind == "ExternalInput":
            if name != part_name:
                in_names.append(name)
        elif alloc.kind == "ExternalOutput":
            np_dt = mybir.dt.np(alloc.dtype)
            out_avals.append(jax.core.ShapedArray(tuple(alloc.tensor_shape), np_dt))
            out_names.append(name)
            zero_like.append((tuple(alloc.tensor_shape), np_dt))
    n_params = len(in_names)
    bind_in_names = tuple(in_names + out_names + ([part_name] if part_name else []))

    def _body(*args):
        ins = list(args[:n_params])
        outs = list(args[n_params:])
        extra = [partition_id_tensor()] if part_name else []
        outs = list(_bass_exec_p.bind(
            *ins, *outs, *extra,
            out_avals=tuple(out_avals),
            in_names=bind_in_names,
            out_names=tuple(out_names),
            lowering_input_output_aliases=(),
            sim_require_finite=True,
            sim_require_nnan=True,
            nc=nc,
        ))
        return tuple(outs)

    devices = jax.devices()[:NCORES]
    mesh = Mesh(np.asarray(devices), ("core",))
    n_outs = len(out_names)
    sharded = jax.jit(
        shard_map(
            _body, mesh=mesh,
            in_specs=(PartitionSpec("core"),) * (n_params + n_outs),
            out_specs=(PartitionSpec("core"),) * n_outs,
            check_rep=False,
        ),
        donate_argnums=tuple(range(n_params, n_params + n_outs)),
        keep_unused=True,
    )

    sh = NamedSharding(mesh, PartitionSpec("core"))
    import jax.numpy as jnp
    zeros_fn = jax.jit(
        lambda: tuple(
            jnp.zeros((NCORES * s[0],) + s[1:], d) for s, d in zero_like
        ),
        out_shardings=(sh,) * n_outs,
    )

    def run(per_core):
        # [8, s0, ...] -> [8*s0, ...] is a reshape view, not a copy
        concat_in = [
            np.ascontiguousarray(per_core[n]).reshape(
                (NCORES * per_core[n].shape[1],) + tuple(per_core[n].shape[2:])
            )
            for n in in_names
        ]
        # donated output buffers created on-device: avoids shipping 32 MB
        # of zeros over the (slow) axon link every call
        zeros = zeros_fn()
        outs = sharded(*concat_in, *zeros)
        res = {}
        for i, name in enumerate(out_names):
            a = np.asarray(outs[i])
            res[name] = a.reshape(NCORES, *out_avals[i].shape)
        return res

    _RUNNER_CACHE[kt] = run
    return run


def _pack_inputs(q, k, v, ratio, scale, attn_mask):
    """Host-side packing into the per-core flat layouts.

    Drops masked keys entirely (they contribute exactly 0 after exp) and
    packs the kept keys contiguously, padded to KT*128 with -1e9-bias
    slots. Sets the module-level _CUR_KT so a subsequent _get_nc() builds
    the matching kernel."""
    global _CUR_KT
    q = np.asarray(q, dtype=np.float32)
    k = np.asarray(k, dtype=np.float32)
    v = np.asarray(v, dtype=np.float32)
    ratio = np.asarray(ratio, dtype=np.float32)
    mask = np.asarray(attn_mask).astype(bool)

    keep = [np.nonzero(~mask[b])[0] for b in range(B)]
    nmax = max(len(ix) for ix in keep)
    KT = max(1, -(-nmax // P))
    if KT > LK // P:
        KT = LK // P
    _CUR_KT = KT
    KTP = KT * P

    mult = np.float32(scale) * ratio  # [B]
    qs = q * mult[:, None, None]      # [B, LQ, D]

    kc = np.zeros((B, KTP, D), dtype=np.float32)
    vc = np.zeros((B, KTP, D), dtype=np.float32)
    bias = np.full((B, KTP), NEG, dtype=np.float32)
    for b in range(B):
        n = len(keep[b])
        kc[b, :n] = k[b, keep[b]]
        vc[b, :n] = v[b, keep[b]]
        bias[b, :n] = 0.0

    # K region: [B, 128(d_in_tile), DT*KTP], col = d*KTP + key
    kd = np.ascontiguousarray(kc.transpose(0, 2, 1)).reshape(B, DT, P, KTP)
    kreg = np.ascontiguousarray(kd.transpose(0, 2, 1, 3)).reshape(B, P, DT * KTP)
    # Q region: col = d*LQ + q
    qd = np.ascontiguousarray(qs.transpose(0, 2, 1)).reshape(B, DT, P, LQ)
    qreg = np.ascontiguousarray(qd.transpose(0, 2, 1, 3)).reshape(B, P, DT * LQ)
    kq = np.concatenate([kreg, qreg], axis=2).astype(NPBF16)  # [B, 128, cols]

    vvl = vc.reshape(B, KT, P, D).transpose(0, 2, 1, 3)
    vvl = np.ascontiguousarray(vvl).reshape(B, P, KT * D).astype(NPBF16)

    # consts[b, p, t] = bias for key t*128+p
    consts = np.ascontiguousarray(
        bias.reshape(B, KT, P).transpose(0, 2, 1)
    )

    ones = np.ones((P, P), dtype=NPBF16)
    return kq, vvl, consts, ones


def kernel(q, k, v, ratio, scale, attn_mask):
    """Full inputs in, full output out. Shards batch across 8 cores."""
    q = np.asarray(q)
    k = np.asarray(k)
    v = np.asarray(v)
    ratio = np.asarray(ratio)
    scale = np.asarray(scale)
    attn_mask = np.asarray(attn_mask)
    assert q.shape == (B, LQ, D) and k.shape == (B, LK, D)
    kq, vvl, consts, ones = _pack_inputs(q, k, v, ratio, scale, attn_mask)
    run = _get_runner(_CUR_KT)
    per_core = {
        "kq": kq, "vv": vvl, "consts": consts,
        "onesd": np.broadcast_to(ones, (B,) + ones.shape),
    }
    res = run(per_core)
    out_un = res["out_u"]                          # [B, LQ, D]
    ssum = res["sums"].reshape(B, LQ)
    out = out_un / ssum[:, :, None]
    return out.astype(np.float32)
